# revision 7
# baseline (speedup 1.0000x reference)
"""GCN layer (4-relation message passing) on 8 Trainium2 NeuronCores.

out = sum_r (A_r @ inp) @ W_r + sum_r b_r,  A_r in COO form (dst, src, val).

Sharding: edges sharded by dst range; core c owns dst in [c*12500, (c+1)*12500).
Host stages per-edge message rows inp[src] into block-ordered slabs (per-core
data layout, like the sharding itself); all arithmetic runs on device:

device, per core, per (64-node dst window w, relation r) cell:
  per 128-edge block b: DVE builds selection matrix O[p, j] = val_p *
  (j == dstloc_p); PE accumulates  aggT_wr [64in, 64nodes] += MSG_b^T O_b
  in PSUM (the val scaling and the dst segment-sum happen inside the matmul).
  Then per window: 4x PE  out_w [64nodes, 64outf] += aggT_wr^T @ W_r.
host: concat core slices, add summed bias.
"""

import math
from contextlib import ExitStack

import numpy as np

import concourse.bass as bass
import concourse.tile as tile
from concourse import bacc, mybir
from concourse.bass_utils import run_bass_kernel_spmd

# problem constants
N_NODES = 100000
N_REL = 4
N_EDGES = 1600000
IN_SIZE = 64
OUT_SIZE = 64

N_CORES = 8
NPC = N_NODES // N_CORES  # nodes (dst) per core
P = 128                   # partitions / edges per block
W = 64                    # dst-window width (nodes per psum tile)
GW = 2                    # windows per message-slab DMA
EDT = "f32"               # message/one-hot dtype: "f32" | "bf16"

F32 = mybir.dt.float32
BF16 = mybir.dt.bfloat16
_NPDT = {"f32": np.float32, "bf16": None}


def _np_edt():
    if EDT == "f32":
        return np.float32
    import ml_dtypes
    return ml_dtypes.bfloat16


def _mb_edt():
    return F32 if EDT == "f32" else BF16


def _host_prep(inp, src, dst, edge_val):
    """Bucket/pad edges per (core, window, rel); build device arrays."""
    n_win = math.ceil(NPC / W)
    ncell = n_win * N_REL
    srcf = src.reshape(-1).astype(np.int64)
    dstf = dst.reshape(-1).astype(np.int64)
    valf = edge_val.reshape(-1).astype(np.float32)
    rel = np.repeat(np.arange(N_REL, dtype=np.int64), src.shape[1])

    core = dstf // NPC
    dloc = dstf % NPC
    win = dloc // W
    wloc = (dloc % W).astype(np.float32)
    cell = win * N_REL + rel                      # per-core cell id
    key = core * ncell + cell

    counts = np.bincount(key, minlength=N_CORES * ncell).reshape(
        N_CORES, ncell)
    B = np.maximum((counts.max(axis=0) + P - 1) // P, 1).astype(np.int64)
    starts = np.zeros(ncell + 1, dtype=np.int64)
    np.cumsum(B, out=starts[1:])
    T = int(starts[-1])

    edt = _np_edt()
    dstw_all = np.zeros((N_CORES, P, T), dtype=edt)
    vals_all = np.zeros((N_CORES, P, T), dtype=edt)
    msg_all = np.zeros((N_CORES, P, T, IN_SIZE), dtype=edt)

    order = np.argsort(key, kind="stable")
    grp_start = np.zeros(N_CORES * ncell, dtype=np.int64)
    np.cumsum(counts.reshape(-1)[:-1], out=grp_start[1:])
    j = np.arange(len(order), dtype=np.int64) - grp_start[key[order]]
    t_col = starts[cell[order]] + (j // P)
    p_row = j % P
    c_ord = core[order]
    dstw_all[c_ord, p_row, t_col] = wloc[order].astype(edt)
    vals_all[c_ord, p_row, t_col] = valf[order].astype(edt)
    msg_all[c_ord, p_row, t_col, :] = inp[srcf[order]].astype(edt)

    return n_win, B, starts, T, dstw_all, vals_all, msg_all


_PROG_CACHE = {}


def _build_program(n_win, starts, T):
    key = (EDT, n_win, tuple(int(s) for s in starts), T)
    if key in _PROG_CACHE:
        return _PROG_CACHE[key]

    EF = _mb_edt()
    nc = bacc.Bacc("TRN2", target_bir_lowering=False, debug=False,
                   num_devices=N_CORES)
    wcat = nc.dram_tensor("wcat", [IN_SIZE, N_REL * OUT_SIZE], F32,
                          kind="ExternalInput").ap()
    edst = nc.dram_tensor("edst", [P, T], EF, kind="ExternalInput").ap()
    evalt = nc.dram_tensor("evalt", [P, T], EF, kind="ExternalInput").ap()
    emsg = nc.dram_tensor("emsg", [P, T * IN_SIZE], EF,
                          kind="ExternalInput").ap()
    iota = nc.dram_tensor("iota", [P, W], EF, kind="ExternalInput").ap()
    n_wcol = (n_win + 1) // 2
    out = nc.dram_tensor("out", [P, n_wcol * OUT_SIZE], F32,
                         kind="ExternalOutput").ap()

    with tile.TileContext(nc) as tc, ExitStack() as ctx:
        p_w = ctx.enter_context(tc.tile_pool(name="p_w", bufs=1))
        p_ed = ctx.enter_context(tc.tile_pool(name="p_ed", bufs=2))
        p_ev = ctx.enter_context(tc.tile_pool(name="p_ev", bufs=2))
        p_msg = ctx.enter_context(tc.tile_pool(name="p_msg", bufs=2))
        p_oh = ctx.enter_context(tc.tile_pool(name="p_oh", bufs=6))
        p_agg = ctx.enter_context(tc.tile_pool(name="p_agg", bufs=6))
        p_c = ctx.enter_context(tc.tile_pool(name="p_c", bufs=1))
        ps_agg = ctx.enter_context(tc.tile_pool(name="ps_agg", bufs=3,
                                                space="PSUM"))
        ps_out = ctx.enter_context(tc.tile_pool(name="ps_out", bufs=2,
                                                space="PSUM"))

        wt = p_w.tile([IN_SIZE, N_REL * OUT_SIZE], F32)
        nc.sync.dma_start(wt[:], wcat[:])
        iot = p_c.tile([P, W], EF)
        nc.sync.dma_start(iot[:], iota[:])
        outsb = p_c.tile([P, n_wcol * OUT_SIZE], F32)
        if n_win % 2:
            nc.vector.memset(outsb[W:P, (n_win // 2) * OUT_SIZE:], 0.0)

        bg_max = max(
            int(starts[min(w0 + GW, n_win) * N_REL] - starts[w0 * N_REL])
            for w0 in range(0, n_win, GW))
        for w0 in range(0, n_win, GW):
            w1 = min(w0 + GW, n_win)
            t0, t1 = int(starts[w0 * N_REL]), int(starts[w1 * N_REL])
            bg = t1 - t0
            dtile = p_ed.tile([P, bg_max], EF, tag="ed")
            nc.sync.dma_start(dtile[:, :bg], edst[:, t0:t1])
            vtile = p_ev.tile([P, bg_max], EF, tag="ev")
            nc.sync.dma_start(vtile[:, :bg], evalt[:, t0:t1])
            mt = p_msg.tile([P, bg_max * IN_SIZE], EF, tag="msg")
            nc.scalar.dma_start(mt[:, :bg * IN_SIZE],
                                emsg[:, t0 * IN_SIZE:t1 * IN_SIZE])
            for w in range(w0, w1):
                aggs = []
                for r in range(N_REL):
                    c2 = w * N_REL + r
                    b0, b1 = int(starts[c2]) - t0, int(starts[c2 + 1]) - t0
                    ps = ps_agg.tile([IN_SIZE, W], F32)
                    for b in range(b0, b1):
                        oh = p_oh.tile([P, W], EF)
                        nc.vector.tensor_scalar(
                            out=oh[:], in0=iot[:],
                            scalar1=dtile[:, b:b + 1],
                            scalar2=vtile[:, b:b + 1],
                            op0=mybir.AluOpType.is_equal,
                            op1=mybir.AluOpType.mult)
                        nc.tensor.matmul(
                            out=ps[:],
                            lhsT=mt[:, b * IN_SIZE:(b + 1) * IN_SIZE],
                            rhs=oh[:],
                            start=(b == b0), stop=(b == b1 - 1))
                    agg = p_agg.tile([IN_SIZE, W], F32, tag="agg")
                    if r % 2 == 0:
                        nc.scalar.copy(agg[:], ps[:])
                    else:
                        nc.vector.tensor_copy(agg[:], ps[:])
                    aggs.append(agg)
                po = ps_out.tile([W, OUT_SIZE], F32)
                for r in range(N_REL):
                    nc.tensor.matmul(
                        out=po[:],
                        lhsT=aggs[r][:],
                        rhs=wt[:, r * OUT_SIZE:(r + 1) * OUT_SIZE],
                        start=(r == 0), stop=(r == N_REL - 1))
                nc.scalar.copy(
                    outsb[(w % 2) * W:(w % 2) * W + W,
                          (w // 2) * OUT_SIZE:(w // 2 + 1) * OUT_SIZE],
                    po[:])
        nc.sync.dma_start(out[:], outsb[:])

    nc.compile()
    _PROG_CACHE[key] = nc
    return nc


def kernel(inp, src, dst, edge_val, weights, bias):
    inp = np.asarray(inp, dtype=np.float32)
    src = np.asarray(src)
    dst = np.asarray(dst)
    edge_val = np.asarray(edge_val, dtype=np.float32)
    weights = np.asarray(weights, dtype=np.float32)
    bias = np.asarray(bias, dtype=np.float32)

    n_win, B, starts, T, dstw_all, vals_all, msg_all = _host_prep(
        inp, src, dst, edge_val)
    nc = _build_program(n_win, starts, T)

    wcat = np.ascontiguousarray(
        weights.transpose(1, 0, 2).reshape(IN_SIZE, N_REL * OUT_SIZE))
    iota = np.tile(np.arange(W, dtype=np.float32), (P, 1)).astype(_np_edt())

    in_maps = []
    for c in range(N_CORES):
        in_maps.append({
            "wcat": wcat,
            "edst": dstw_all[c],
            "evalt": vals_all[c],
            "emsg": msg_all[c].reshape(P, T * IN_SIZE),
            "iota": iota,
        })
    res = run_bass_kernel_spmd(nc, in_maps, list(range(N_CORES)))
    # decode outsb layout: row (w%2)*64+j, col-block w//2 -> node w*64+j
    n_wcol = (n_win + 1) // 2
    parts = []
    for c in range(N_CORES):
        arr = res.results[c]["out"].reshape(2, W, n_wcol, OUT_SIZE)
        nodes = arr.transpose(2, 0, 1, 3).reshape(n_wcol * 2 * W, OUT_SIZE)
        parts.append(nodes[:NPC])
    out = np.concatenate(parts, axis=0)
    out = out + bias.sum(axis=0)
    return out.astype(np.float32)


# revision 9
# speedup vs baseline: 1.1351x; 1.1351x over previous
"""GCN layer (4-relation message passing) on 8 Trainium2 NeuronCores.

out = sum_r (A_r @ inp) @ W_r + sum_r b_r,  A_r in COO form (dst, src, val).

Sharding: edges sharded by dst range; core c owns dst in [c*12500, (c+1)*12500).
Host stages per-edge message rows inp[src] into block-ordered slabs (per-core
data layout, like the sharding itself); all arithmetic runs on device:

device, per core, per (64-node dst window w, relation r) cell:
  per 128-edge block b: DVE builds selection matrix O[p, j] = val_p *
  (j == dstloc_p); PE accumulates  aggT_wr [64in, 64nodes] += MSG_b^T O_b
  in PSUM (the val scaling and the dst segment-sum happen inside the matmul).
  Then per window: 4x PE  out_w [64nodes, 64outf] += aggT_wr^T @ W_r.
host: concat core slices, add summed bias.
"""

import math
from contextlib import ExitStack

import numpy as np

import concourse.bass as bass
import concourse.tile as tile
from concourse import bacc, mybir
from concourse.bass_utils import run_bass_kernel_spmd

# problem constants
N_NODES = 100000
N_REL = 4
N_EDGES = 1600000
IN_SIZE = 64
OUT_SIZE = 64

N_CORES = 8
NPC = N_NODES // N_CORES  # nodes (dst) per core
P = 128                   # partitions / edges per block
W = 64                    # dst-window width (nodes per psum tile)
GW = 2                    # windows per message-slab DMA
EDT = "bf16"               # message/one-hot dtype: "f32" | "bf16"

F32 = mybir.dt.float32
BF16 = mybir.dt.bfloat16
_NPDT = {"f32": np.float32, "bf16": None}


def _np_edt():
    if EDT == "f32":
        return np.float32
    import ml_dtypes
    return ml_dtypes.bfloat16


def _mb_edt():
    return F32 if EDT == "f32" else BF16


def _host_prep(inp, src, dst, edge_val):
    """Bucket/pad edges per (core, window, rel); build device arrays."""
    n_win = math.ceil(NPC / W)
    ncell = n_win * N_REL
    srcf = src.reshape(-1).astype(np.int64)
    dstf = dst.reshape(-1).astype(np.int64)
    valf = edge_val.reshape(-1).astype(np.float32)
    rel = np.repeat(np.arange(N_REL, dtype=np.int64), src.shape[1])

    core = dstf // NPC
    dloc = dstf % NPC
    win = dloc // W
    wloc = (dloc % W).astype(np.float32)
    cell = win * N_REL + rel                      # per-core cell id
    key = core * ncell + cell

    counts = np.bincount(key, minlength=N_CORES * ncell).reshape(
        N_CORES, ncell)
    B = np.maximum((counts.max(axis=0) + P - 1) // P, 1).astype(np.int64)
    starts = np.zeros(ncell + 1, dtype=np.int64)
    np.cumsum(B, out=starts[1:])
    T = int(starts[-1])

    edt = _np_edt()
    dstw_all = np.zeros((N_CORES, P, T), dtype=np.float32)
    vals_all = np.zeros((N_CORES, P, T), dtype=np.float32)
    msg_all = np.zeros((N_CORES, P, T, IN_SIZE), dtype=edt)

    order = np.argsort(key, kind="stable")
    grp_start = np.zeros(N_CORES * ncell, dtype=np.int64)
    np.cumsum(counts.reshape(-1)[:-1], out=grp_start[1:])
    j = np.arange(len(order), dtype=np.int64) - grp_start[key[order]]
    t_col = starts[cell[order]] + (j // P)
    p_row = j % P
    c_ord = core[order]
    dstw_all[c_ord, p_row, t_col] = wloc[order]
    vals_all[c_ord, p_row, t_col] = valf[order]
    msg_all[c_ord, p_row, t_col, :] = inp[srcf[order]].astype(edt)

    return n_win, B, starts, T, dstw_all, vals_all, msg_all


_PROG_CACHE = {}


def _build_program(n_win, starts, T):
    key = (EDT, n_win, tuple(int(s) for s in starts), T)
    if key in _PROG_CACHE:
        return _PROG_CACHE[key]

    EF = _mb_edt()
    nc = bacc.Bacc("TRN2", target_bir_lowering=False, debug=False,
                   num_devices=N_CORES)
    wcat = nc.dram_tensor("wcat", [IN_SIZE, N_REL * OUT_SIZE], F32,
                          kind="ExternalInput").ap()
    edst = nc.dram_tensor("edst", [P, T], F32, kind="ExternalInput").ap()
    evalt = nc.dram_tensor("evalt", [P, T], F32, kind="ExternalInput").ap()
    emsg = nc.dram_tensor("emsg", [P, T * IN_SIZE], EF,
                          kind="ExternalInput").ap()
    iota = nc.dram_tensor("iota", [P, W], EF, kind="ExternalInput").ap()
    n_wcol = (n_win + 1) // 2
    out = nc.dram_tensor("out", [P, n_wcol * OUT_SIZE], F32,
                         kind="ExternalOutput").ap()

    with tile.TileContext(nc) as tc, ExitStack() as ctx:
        p_w = ctx.enter_context(tc.tile_pool(name="p_w", bufs=1))
        p_ed = ctx.enter_context(tc.tile_pool(name="p_ed", bufs=2))
        p_ev = ctx.enter_context(tc.tile_pool(name="p_ev", bufs=2))
        p_msg = ctx.enter_context(tc.tile_pool(name="p_msg", bufs=2))
        p_oh = ctx.enter_context(tc.tile_pool(name="p_oh", bufs=6))
        p_agg = ctx.enter_context(tc.tile_pool(name="p_agg", bufs=6))
        p_c = ctx.enter_context(tc.tile_pool(name="p_c", bufs=1))
        ps_agg = ctx.enter_context(tc.tile_pool(name="ps_agg", bufs=3,
                                                space="PSUM"))
        ps_out = ctx.enter_context(tc.tile_pool(name="ps_out", bufs=2,
                                                space="PSUM"))

        wt = p_w.tile([IN_SIZE, N_REL * OUT_SIZE], F32)
        nc.sync.dma_start(wt[:], wcat[:])
        iot = p_c.tile([P, W], EF)
        nc.sync.dma_start(iot[:], iota[:])
        outsb = p_c.tile([P, n_wcol * OUT_SIZE], F32)
        if n_win % 2:
            nc.vector.memset(outsb[W:P, (n_win // 2) * OUT_SIZE:], 0.0)

        bg_max = max(
            int(starts[min(w0 + GW, n_win) * N_REL] - starts[w0 * N_REL])
            for w0 in range(0, n_win, GW))
        for w0 in range(0, n_win, GW):
            w1 = min(w0 + GW, n_win)
            t0, t1 = int(starts[w0 * N_REL]), int(starts[w1 * N_REL])
            bg = t1 - t0
            dtile = p_ed.tile([P, bg_max], F32, tag="ed")
            nc.sync.dma_start(dtile[:, :bg], edst[:, t0:t1])
            vtile = p_ev.tile([P, bg_max], F32, tag="ev")
            nc.sync.dma_start(vtile[:, :bg], evalt[:, t0:t1])
            mt = p_msg.tile([P, bg_max * IN_SIZE], EF, tag="msg")
            nc.scalar.dma_start(mt[:, :bg * IN_SIZE],
                                emsg[:, t0 * IN_SIZE:t1 * IN_SIZE])
            for w in range(w0, w1):
                aggs = []
                for r in range(N_REL):
                    c2 = w * N_REL + r
                    b0, b1 = int(starts[c2]) - t0, int(starts[c2 + 1]) - t0
                    ps = ps_agg.tile([IN_SIZE, W], F32)
                    for b in range(b0, b1):
                        oh = p_oh.tile([P, W], EF)
                        nc.vector.tensor_scalar(
                            out=oh[:], in0=iot[:],
                            scalar1=dtile[:, b:b + 1],
                            scalar2=vtile[:, b:b + 1],
                            op0=mybir.AluOpType.is_equal,
                            op1=mybir.AluOpType.mult)
                        nc.tensor.matmul(
                            out=ps[:],
                            lhsT=mt[:, b * IN_SIZE:(b + 1) * IN_SIZE],
                            rhs=oh[:],
                            start=(b == b0), stop=(b == b1 - 1))
                    agg = p_agg.tile([IN_SIZE, W], F32, tag="agg")
                    if r % 2 == 0:
                        nc.scalar.copy(agg[:], ps[:])
                    else:
                        nc.vector.tensor_copy(agg[:], ps[:])
                    aggs.append(agg)
                po = ps_out.tile([W, OUT_SIZE], F32)
                for r in range(N_REL):
                    nc.tensor.matmul(
                        out=po[:],
                        lhsT=aggs[r][:],
                        rhs=wt[:, r * OUT_SIZE:(r + 1) * OUT_SIZE],
                        start=(r == 0), stop=(r == N_REL - 1))
                nc.scalar.copy(
                    outsb[(w % 2) * W:(w % 2) * W + W,
                          (w // 2) * OUT_SIZE:(w // 2 + 1) * OUT_SIZE],
                    po[:])
        nc.sync.dma_start(out[:], outsb[:])

    nc.compile()
    _PROG_CACHE[key] = nc
    return nc


def kernel(inp, src, dst, edge_val, weights, bias):
    inp = np.asarray(inp, dtype=np.float32)
    src = np.asarray(src)
    dst = np.asarray(dst)
    edge_val = np.asarray(edge_val, dtype=np.float32)
    weights = np.asarray(weights, dtype=np.float32)
    bias = np.asarray(bias, dtype=np.float32)

    n_win, B, starts, T, dstw_all, vals_all, msg_all = _host_prep(
        inp, src, dst, edge_val)
    nc = _build_program(n_win, starts, T)

    wcat = np.ascontiguousarray(
        weights.transpose(1, 0, 2).reshape(IN_SIZE, N_REL * OUT_SIZE))
    iota = np.tile(np.arange(W, dtype=np.float32), (P, 1)).astype(_np_edt())

    in_maps = []
    for c in range(N_CORES):
        in_maps.append({
            "wcat": wcat,
            "edst": dstw_all[c],
            "evalt": vals_all[c],
            "emsg": msg_all[c].reshape(P, T * IN_SIZE),
            "iota": iota,
        })
    res = run_bass_kernel_spmd(nc, in_maps, list(range(N_CORES)))
    # decode outsb layout: row (w%2)*64+j, col-block w//2 -> node w*64+j
    n_wcol = (n_win + 1) // 2
    parts = []
    for c in range(N_CORES):
        arr = res.results[c]["out"].reshape(2, W, n_wcol, OUT_SIZE)
        nodes = arr.transpose(2, 0, 1, 3).reshape(n_wcol * 2 * W, OUT_SIZE)
        parts.append(nodes[:NPC])
    out = np.concatenate(parts, axis=0)
    out = out + bias.sum(axis=0)
    return out.astype(np.float32)


# revision 10
# speedup vs baseline: 2.5544x; 2.2503x over previous
"""GCN layer (4-relation message passing) on 8 Trainium2 NeuronCores.

out = sum_r (A_r @ inp) @ W_r + sum_r b_r,  A_r in COO form (dst, src, val).

Sharding: edges sharded by dst range; core c owns dst in [c*12500, (c+1)*12500).
Host stages, per 128-edge block, a [128, 128] slab: columns 0:64 hold the
message rows inp[src], columns 64:128 hold the selection matrix
O[p, j] = val_p * (j == dstloc_p)  (pure placement of input values - no
host arithmetic). Device does all FLOPs:

per (64-node dst window w, relation r) cell, per block b:
  PE accumulates  aggT_wr [64in, 64nodes] += MSG_b^T @ O_b   in PSUM
  (edge_val scaling and the dst segment-sum happen inside this matmul).
Per window: 4x PE  out_w [64nodes, 64outf] += aggT_wr^T @ W_r.
Host: concat core slices, add summed bias.
"""

import math
from contextlib import ExitStack

import numpy as np

import concourse.bass as bass
import concourse.tile as tile
from concourse import bacc, mybir
from concourse.bass_utils import run_bass_kernel_spmd

# problem constants
N_NODES = 100000
N_REL = 4
N_EDGES = 1600000
IN_SIZE = 64
OUT_SIZE = 64

N_CORES = 8
NPC = N_NODES // N_CORES  # nodes (dst) per core
P = 128                   # partitions / edges per block
W = 64                    # dst-window width (nodes per psum tile)
BW = IN_SIZE + W          # block slab width (msg cols + selection cols)
GW = 2                    # windows per slab DMA
EDT = "bf16"              # message/selection dtype: "f32" | "bf16"

F32 = mybir.dt.float32
BF16 = mybir.dt.bfloat16


def _np_edt():
    if EDT == "f32":
        return np.float32
    import ml_dtypes
    return ml_dtypes.bfloat16


def _mb_edt():
    return F32 if EDT == "f32" else BF16


def _host_prep(inp, src, dst, edge_val):
    """Bucket/pad edges per (core, window, rel); build block slabs."""
    n_win = math.ceil(NPC / W)
    ncell = n_win * N_REL
    srcf = src.reshape(-1).astype(np.int64)
    dstf = dst.reshape(-1).astype(np.int64)
    valf = edge_val.reshape(-1).astype(np.float32)
    rel = np.repeat(np.arange(N_REL, dtype=np.int64), src.shape[1])

    core = dstf // NPC
    dloc = dstf % NPC
    win = dloc // W
    wloc = dloc % W
    cell = win * N_REL + rel
    key = core * ncell + cell

    counts = np.bincount(key, minlength=N_CORES * ncell).reshape(
        N_CORES, ncell)
    B = np.maximum((counts.max(axis=0) + P - 1) // P, 1).astype(np.int64)
    starts = np.zeros(ncell + 1, dtype=np.int64)
    np.cumsum(B, out=starts[1:])
    T = int(starts[-1])

    edt = _np_edt()
    slab_all = np.zeros((N_CORES, P, T, BW), dtype=edt)

    order = np.argsort(key, kind="stable")
    grp_start = np.zeros(N_CORES * ncell, dtype=np.int64)
    np.cumsum(counts.reshape(-1)[:-1], out=grp_start[1:])
    j = np.arange(len(order), dtype=np.int64) - grp_start[key[order]]
    t_col = starts[cell[order]] + (j // P)
    p_row = j % P
    c_ord = core[order]
    slab_all[c_ord, p_row, t_col, :IN_SIZE] = inp[srcf[order]].astype(edt)
    slab_all[c_ord, p_row, t_col, IN_SIZE + wloc[order]] = valf[order].astype(edt)

    return n_win, B, starts, T, slab_all


_PROG_CACHE = {}


def _build_program(n_win, starts, T):
    key = (EDT, n_win, tuple(int(s) for s in starts), T)
    if key in _PROG_CACHE:
        return _PROG_CACHE[key]

    EF = _mb_edt()
    nc = bacc.Bacc("TRN2", target_bir_lowering=False, debug=False,
                   num_devices=N_CORES)
    wcat = nc.dram_tensor("wcat", [IN_SIZE, N_REL * OUT_SIZE], F32,
                          kind="ExternalInput").ap()
    eslab = nc.dram_tensor("eslab", [P, T * BW], EF,
                           kind="ExternalInput").ap()
    n_wcol = (n_win + 1) // 2
    out = nc.dram_tensor("out", [P, n_wcol * OUT_SIZE], F32,
                         kind="ExternalOutput").ap()

    with tile.TileContext(nc) as tc, ExitStack() as ctx:
        p_w = ctx.enter_context(tc.tile_pool(name="p_w", bufs=1))
        p_msg = ctx.enter_context(tc.tile_pool(name="p_msg", bufs=3))
        p_agg = ctx.enter_context(tc.tile_pool(name="p_agg", bufs=6))
        p_c = ctx.enter_context(tc.tile_pool(name="p_c", bufs=1))
        ps_agg = ctx.enter_context(tc.tile_pool(name="ps_agg", bufs=3,
                                                space="PSUM"))
        ps_out = ctx.enter_context(tc.tile_pool(name="ps_out", bufs=2,
                                                space="PSUM"))

        wt = p_w.tile([IN_SIZE, N_REL * OUT_SIZE], F32)
        nc.sync.dma_start(wt[:], wcat[:])
        outsb = p_c.tile([P, n_wcol * OUT_SIZE], F32)
        if n_win % 2:
            nc.vector.memset(outsb[W:P, (n_win // 2) * OUT_SIZE:], 0.0)

        bg_max = max(
            int(starts[min(w0 + GW, n_win) * N_REL] - starts[w0 * N_REL])
            for w0 in range(0, n_win, GW))
        for w0 in range(0, n_win, GW):
            w1 = min(w0 + GW, n_win)
            t0, t1 = int(starts[w0 * N_REL]), int(starts[w1 * N_REL])
            bg = t1 - t0
            mt = p_msg.tile([P, bg_max * BW], EF, tag="msg")
            nc.scalar.dma_start(mt[:, :bg * BW],
                                eslab[:, t0 * BW:t1 * BW])
            for w in range(w0, w1):
                aggs = []
                for r in range(N_REL):
                    c2 = w * N_REL + r
                    b0, b1 = int(starts[c2]) - t0, int(starts[c2 + 1]) - t0
                    ps = ps_agg.tile([IN_SIZE, W], F32)
                    for b in range(b0, b1):
                        nc.tensor.matmul(
                            out=ps[:],
                            lhsT=mt[:, b * BW:b * BW + IN_SIZE],
                            rhs=mt[:, b * BW + IN_SIZE:(b + 1) * BW],
                            start=(b == b0), stop=(b == b1 - 1))
                    agg = p_agg.tile([IN_SIZE, W], F32, tag="agg")
                    if r % 2 == 0:
                        nc.scalar.copy(agg[:], ps[:])
                    else:
                        nc.vector.tensor_copy(agg[:], ps[:])
                    aggs.append(agg)
                po = ps_out.tile([W, OUT_SIZE], F32)
                for r in range(N_REL):
                    nc.tensor.matmul(
                        out=po[:],
                        lhsT=aggs[r][:],
                        rhs=wt[:, r * OUT_SIZE:(r + 1) * OUT_SIZE],
                        start=(r == 0), stop=(r == N_REL - 1))
                nc.scalar.copy(
                    outsb[(w % 2) * W:(w % 2) * W + W,
                          (w // 2) * OUT_SIZE:(w // 2 + 1) * OUT_SIZE],
                    po[:])
        nc.sync.dma_start(out[:], outsb[:])

    nc.compile()
    _PROG_CACHE[key] = nc
    return nc


def kernel(inp, src, dst, edge_val, weights, bias):
    inp = np.asarray(inp, dtype=np.float32)
    src = np.asarray(src)
    dst = np.asarray(dst)
    edge_val = np.asarray(edge_val, dtype=np.float32)
    weights = np.asarray(weights, dtype=np.float32)
    bias = np.asarray(bias, dtype=np.float32)

    n_win, B, starts, T, slab_all = _host_prep(inp, src, dst, edge_val)
    nc = _build_program(n_win, starts, T)

    wcat = np.ascontiguousarray(
        weights.transpose(1, 0, 2).reshape(IN_SIZE, N_REL * OUT_SIZE))

    in_maps = []
    for c in range(N_CORES):
        in_maps.append({
            "wcat": wcat,
            "eslab": slab_all[c].reshape(P, T * BW),
        })
    res = run_bass_kernel_spmd(nc, in_maps, list(range(N_CORES)))
    n_wcol = (n_win + 1) // 2
    parts = []
    for c in range(N_CORES):
        arr = res.results[c]["out"].reshape(2, W, n_wcol, OUT_SIZE)
        nodes = arr.transpose(2, 0, 1, 3).reshape(n_wcol * 2 * W, OUT_SIZE)
        parts.append(nodes[:NPC])
    out = np.concatenate(parts, axis=0)
    out = out + bias.sum(axis=0)
    return out.astype(np.float32)
